# revision 10
# baseline (speedup 1.0000x reference)
"""Trainium2 Bass kernel for single-query causal multi-head attention decode.

Math (per batch b):
    q = query @ Wq.T + bq                         [NHID]
    scores[h, l] = k[l, h, :] . q[h, :]   with k = key_cache @ Wk.T + bk
                 = sum_c key_cache[l, c] * qk[h, c]   (+ const that cancels in softmax)
      where qk[h, c] = sum_d Wk[h*D+d, c] * q[h*D+d]   (Wk folded into the query)
    attn = softmax(scores * SCALE) over l <= position_i  (others exactly 0)
    ctx[h, c] = sum_l attn[h, l] * value_cache[l, c]
    out[hd]   = sum_c ctx[h, c] * Wv[hd, c] + bv[hd]   (softmax weights sum to 1)
    y = out @ Wo.T + bo

This avoids projecting the whole K/V cache (the dominant FLOPs of the naive
form); the kernel is HBM-bandwidth-bound streaming the caches once.

Sharding: batch-parallel, 4 batches per core across 8 cores. Weights
replicated. Only cache rows [0, ceil((position_i+1)/128)*128) are read;
the tail of the last 128-row tile is masked to -1e4 before softmax so its
exp underflows to exactly 0.
"""

import math
import sys

import numpy as np

sys.path.insert(0, "/opt/trn_rl_repo")

import concourse.bass as bass  # noqa: E402
import concourse.mybir as mybir  # noqa: E402
import concourse.tile as tile  # noqa: E402
from concourse import bacc  # noqa: E402
from concourse.bass import ds, ts  # noqa: E402
from concourse.masks import make_identity  # noqa: E402

P = 128
NHID = 1024
H = 16
D = NHID // H  # 64
CO = NHID // P  # 8
BL = 4  # batches per core
NCORES = 8
BATCH = BL * NCORES
CACHE_LEN = 4096
SCALE = 1.0 / math.sqrt(D)
NEG_BIG = -1.0e4  # masked logit; exp((NEG_BIG - max) * SCALE) underflows to 0.0
F32 = mybir.dt.float32
AF = mybir.ActivationFunctionType


def build_nc(L_pad: int, Leff: int, use_f32r: bool = True):
    """Build the per-core Bass program. All 8 cores run the identical NEFF on
    their own 4-batch shard."""
    n_lt = L_pad // P
    DT = mybir.dt.float32r if use_f32r else F32

    nc = bacc.Bacc(
        "TRN2", target_bir_lowering=False, debug=False, enable_asserts=False
    )

    q_d = nc.dram_tensor("query", (BL, NHID), DT, kind="ExternalInput").ap()
    kc_d = nc.dram_tensor("key_cache", (BL, L_pad, NHID), DT, kind="ExternalInput").ap()
    vc_d = nc.dram_tensor(
        "value_cache", (BL, L_pad, NHID), DT, kind="ExternalInput"
    ).ap()
    wq_d = nc.dram_tensor("Wq", (NHID, NHID), DT, kind="ExternalInput").ap()
    wk_d = nc.dram_tensor("Wk", (NHID, NHID), DT, kind="ExternalInput").ap()
    wv_d = nc.dram_tensor("Wv", (NHID, NHID), DT, kind="ExternalInput").ap()
    wo_d = nc.dram_tensor("Wo", (NHID, NHID), DT, kind="ExternalInput").ap()
    bq_d = nc.dram_tensor("bq", (NHID,), F32, kind="ExternalInput").ap()
    bv_d = nc.dram_tensor("bv", (NHID,), F32, kind="ExternalInput").ap()
    bo_d = nc.dram_tensor("bo", (NHID,), F32, kind="ExternalInput").ap()
    y_d = nc.dram_tensor("out", (BL, NHID), F32, kind="ExternalOutput").ap()

    # l-tile groups of up to 4 tiles (512 columns of scores per group)
    groups = []
    t = 0
    while t < n_lt:
        gl = min(4, n_lt - t)
        groups.append((t, gl))
        t += gl

    with tile.TileContext(nc) as tc:
        with (
            tc.tile_pool(name="persist", bufs=1) as persist,
            tc.tile_pool(name="ps_big", bufs=2, space="PSUM") as ps_big,
            tc.tile_pool(name="ps_med", bufs=2, space="PSUM") as ps_med,
            tc.tile_pool(name="ps_ctx", bufs=2, space="PSUM") as ps_ctx,
        ):
            ident = persist.tile([P, P], DT)
            make_identity(nc, ident)
            # qk, folded key weights applied to q:  [c_in, c_chunk, h, b]
            qkT = persist.tile([P, CO, H, BL], DT)
            # normalized context transposed:        [c_in, c_chunk, b, h]
            ctxT = persist.tile([P, CO, BL, H], DT)
            bv_sb = persist.tile([BL, NHID], F32)
            nc.gpsimd.dma_start(bv_sb, bv_d[None, :].to_broadcast((BL, NHID)))
            bo_sb = persist.tile([BL, NHID], F32)
            nc.gpsimd.dma_start(bo_sb, bo_d[None, :].to_broadcast((BL, NHID)))
            xT = persist.tile([P, CO, BL], DT)  # query transposed [e, b]
            qT = persist.tile([P, CO, BL], DT)  # q transposed [hd, b]
            o1T = persist.tile([P, CO, BL], DT)  # attn-output transposed [hd, b]

            def transpose_128(ps_slice, in_ap, start=True, stop=True):
                kp = in_ap.partition_size()
                nc.tensor.matmul(
                    ps_slice,
                    in_ap,
                    ident[:kp, :kp],
                    is_transpose=True,
                    start=start,
                    stop=stop,
                )

            def small_transposes(src_sb, dst):
                # src_sb [BL, NHID] -> dst [P, CO, BL]
                for half in range(2):
                    ps = ps_big.tile([P, 4, P], DT, tag="bigT")
                    for jj in range(4):
                        j = half * 4 + jj
                        transpose_128(
                            ps[:, jj, :BL],
                            src_sb[:BL, ts(j, P)],
                            start=(jj == 0),
                            stop=(jj == 3),
                        )
                    nc.scalar.copy(dst[:, ds(half * 4, 4), :], ps[:, :, :BL])

            # Batch-loop pools opened before phase-0 pools so phase-0 scratch
            # stacks above them and releases cleanly before the main loop.
            with (
                tc.tile_pool(name="kc", bufs=4) as kc_pool,
                tc.tile_pool(name="vc", bufs=6) as vc_pool,
                tc.tile_pool(name="kcT", bufs=2) as kcT_pool,
                tc.tile_pool(name="sm", bufs=2) as sm,
                tc.tile_pool(name="aT", bufs=2) as aT_pool,
            ):
                # ---------------- phase 0: q, qk ----------------
                with (
                    tc.tile_pool(name="ph0w", bufs=1) as ph0w,
                    tc.tile_pool(name="ph0s", bufs=2) as ph0s,
                ):
                    x_sb = ph0s.tile([BL, NHID], DT, tag="x")
                    nc.sync.dma_start(x_sb, q_d)
                    small_transposes(x_sb, xT)

                    wq_sb = ph0w.tile([P, CO, NHID], DT, tag="w_nat")
                    nc.sync.dma_start(wq_sb, wq_d.rearrange("(o p) e -> p o e", p=P))
                    bq_sb = ph0s.tile([BL, NHID], F32, tag="bias")
                    nc.gpsimd.dma_start(bq_sb, bq_d[None, :].to_broadcast((BL, NHID)))

                    # q = x @ Wq.T (+ bq): accumulate over e-chunks j
                    psq0 = ps_med.tile([BL, 512], F32, tag="med")
                    psq1 = ps_med.tile([BL, 512], F32, tag="med")
                    for j in range(CO):
                        wqT_j = ph0s.tile([P, NHID], DT, tag="wT_j")
                        for half in range(2):
                            ps = ps_big.tile([P, 4, P], DT, tag="bigT")
                            for ii in range(4):
                                i = half * 4 + ii
                                transpose_128(
                                    ps[:, ii, :],
                                    wq_sb[:, i, ts(j, P)],
                                    start=(ii == 0),
                                    stop=(ii == 3),
                                )
                            nc.vector.tensor_copy(
                                wqT_j[:, ds(half * 512, 512)], ps
                            )
                        nc.tensor.matmul(
                            psq0,
                            xT[:, j, :],
                            wqT_j[:, 0:512],
                            start=(j == 0),
                            stop=(j == CO - 1),
                        )
                        nc.tensor.matmul(
                            psq1,
                            xT[:, j, :],
                            wqT_j[:, 512:],
                            start=(j == 0),
                            stop=(j == CO - 1),
                        )
                    q_sb = ph0s.tile([BL, NHID], DT, tag="x")
                    nc.vector.tensor_add(
                        q_sb[:, 0:512], psq0, bq_sb[:, 0:512]
                    )
                    nc.vector.tensor_add(
                        q_sb[:, 512:], psq1, bq_sb[:, 512:]
                    )
                    small_transposes(q_sb, qT)

                    # qk[c, h, b] = sum_d Wk[h*D+d, c] * q[b, h*D+d]
                    wk_sb = ph0w.tile([P, CO, NHID], DT, tag="w_nat")
                    nc.sync.dma_start(wk_sb, wk_d.rearrange("(o p) e -> p o e", p=P))
                    for k in range(CO):
                        psk = ps_med.tile([P, H, BL], F32, tag="med")
                        for h in range(H):
                            o, r = h // 2, (h % 2) * 64
                            nc.tensor.matmul(
                                psk[:, h, :],
                                wk_sb[r : r + 64, o, ts(k, P)],
                                qT[r : r + 64, o, :],
                                start=(h == 0),
                                stop=(h == H - 1),
                            )
                        nc.scalar.copy(qkT[:, k], psk)

                # ---------------- main loop over batches ----------------
                for b in range(BL):
                    scores_sb = sm.tile([H, L_pad], F32, tag="scores")
                    for t0, gl in groups:
                        kcT_g = kcT_pool.tile([P, CO, 512], DT, tag="kcT")
                        for tt in range(gl):
                            t = t0 + tt
                            kc_t = kc_pool.tile([P, NHID], DT, tag="kc")
                            nc.sync.dma_start(kc_t, kc_ds[b][ts(t, P), :])
                            ps_t = ps_big.tile([P, CO, P], DT, tag="bigT")
                            for k in range(CO):
                                transpose_128(
                                    ps_t[:, k, :],
                                    kc_t[:, ts(k, P)],
                                    start=(k % 4 == 0),
                                    stop=(k % 4 == 3),
                                )
                            nc.vector.tensor_copy(kcT_g[:, :, ts(tt, P)], ps_t)
                        gcols = gl * P
                        ps_s = ps_med.tile([H, 512], F32, tag="med")
                        for k in range(CO):
                            nc.tensor.matmul(
                                ps_s[:, :gcols],
                                qkT[:, k, :, b],
                                kcT_g[:, k, :gcols],
                                start=(k == 0),
                                stop=(k == CO - 1),
                            )
                        nc.scalar.copy(scores_sb[:, ds(t0 * P, gcols)], ps_s[:, :gcols])

                    # mask the padded tail, then softmax along the free dim
                    if Leff < L_pad:
                        nc.vector.memset(scores_sb[:, ds(Leff, L_pad - Leff)], NEG_BIG)
                    mx = sm.tile([H, 1], F32, tag="mx")
                    nc.vector.reduce_max(mx, scores_sb, axis=mybir.AxisListType.X)
                    nbias = sm.tile([H, 1], F32, tag="nb")
                    nc.vector.tensor_scalar_mul(nbias, mx, -SCALE)
                    attn_sb = sm.tile([H, L_pad], DT, tag="attn")
                    den = sm.tile([H, 1], F32, tag="den")
                    nc.scalar.activation(
                        attn_sb,
                        scores_sb,
                        AF.Exp,
                        bias=nbias,
                        scale=SCALE,
                        accum_out=den,
                    )
                    rden = sm.tile([H, 1], F32, tag="rden")
                    nc.vector.reciprocal(rden, den)

                    # attn transposed to [l, h] for the V-side matmul
                    aT = aT_pool.tile([P, n_lt, H], DT, tag="aT")
                    for t0 in range(0, n_lt, 4):
                        cnt = min(4, n_lt - t0)
                        ps_a = ps_big.tile([P, 4, P], DT, tag="bigT")
                        for tt in range(cnt):
                            transpose_128(
                                ps_a[:, tt, :H],
                                attn_sb[:, ts(t0 + tt, P)],
                                start=(tt == 0),
                                stop=(tt == cnt - 1),
                            )
                        nc.scalar.copy(aT[:, ds(t0, cnt), :], ps_a[:, 0:cnt, :H])

                    # ctx[h, c] = sum_l attn[h, l] * vc[l, c]  (unnormalized)
                    ctx0 = ps_ctx.tile([H, 512], F32, tag="ctx")
                    ctx1 = ps_ctx.tile([H, 512], F32, tag="ctx")
                    for t in range(n_lt):
                        vc_t = vc_pool.tile([P, NHID], DT, tag="vc")
                        nc.sync.dma_start(vc_t, vc_ds[b][ts(t, P), :])
                        st, sp = (t == 0), (t == n_lt - 1)
                        nc.tensor.matmul(
                            ctx0, aT[:, t, :], vc_t[:, 0:512], start=st, stop=sp
                        )
                        nc.tensor.matmul(
                            ctx1, aT[:, t, :], vc_t[:, 512:], start=st, stop=sp
                        )
                    ctxn = sm.tile([H, NHID], DT, tag="ctxn")
                    nc.vector.tensor_scalar_mul(ctxn[:, 0:512], ctx0, rden)
                    nc.vector.tensor_scalar_mul(ctxn[:, 512:], ctx1, rden)
                    for half in range(2):
                        ps_c = ps_big.tile([P, 4, P], DT, tag="bigT")
                        for jj in range(4):
                            j = half * 4 + jj
                            transpose_128(
                                ps_c[:, jj, :H],
                                ctxn[:, ts(j, P)],
                                start=(jj == 0),
                                stop=(jj == 3),
                            )
                        nc.scalar.copy(
                            ctxT[:, ds(half * 4, 4), b, :], ps_c[:, :, :H]
                        )

            # ---------------- output projections ----------------
            with (
                tc.tile_pool(name="phfw", bufs=2) as phfw,
                tc.tile_pool(name="phfs", bufs=2) as phfs,
            ):
                wv_sb = phfw.tile([P, CO, NHID], DT, tag="w_nat")
                nc.sync.dma_start(wv_sb, wv_d.rearrange("(o p) e -> p o e", p=P))
                # o1[b, hd] = sum_c ctx[b, h, c] * Wv[hd, c]
                ps_o = ps_big.tile([BL, NHID], F32, tag="bigT")
                for k in range(CO):
                    wvT_k = phfs.tile([P, NHID], DT, tag="wT_k")
                    for half in range(2):
                        ps = ps_big.tile([P, 4, P], DT, tag="bigT")
                        for ii in range(4):
                            i = half * 4 + ii
                            transpose_128(
                                ps[:, ii, :],
                                wv_sb[:, i, ts(k, P)],
                                start=(ii == 0),
                                stop=(ii == 3),
                            )
                        nc.vector.tensor_copy(
                            wvT_k[:, ds(half * 512, 512)], ps
                        )
                    for h in range(H):
                        nc.tensor.matmul(
                            ps_o[:, ds(h * D, D)],
                            ctxT[:, k, :, h],
                            wvT_k[:, ds(h * D, D)],
                            start=(k == 0 and h % 8 == 0),
                            stop=(k == CO - 1 and h % 8 == 7),
                        )
                o1_sb = phfs.tile([BL, NHID], DT, tag="small")
                nc.vector.tensor_add(
                    o1_sb, ps_o, bv_sb
                )
                small_transposes(o1_sb, o1T)

                # y = o1 @ Wo.T + bo
                wo_sb = phfw.tile([P, CO, NHID], DT, tag="w_nat")
                nc.sync.dma_start(wo_sb, wo_d.rearrange("(o p) e -> p o e", p=P))
                ps_y = ps_big.tile([BL, NHID], F32, tag="bigT")
                for j in range(CO):
                    woT_j = phfs.tile([P, NHID], DT, tag="wT_k")
                    for half in range(2):
                        ps = ps_big.tile([P, 4, P], DT, tag="bigT")
                        for ii in range(4):
                            i = half * 4 + ii
                            transpose_128(
                                ps[:, ii, :],
                                wo_sb[:, i, ts(j, P)],
                                start=(ii == 0),
                                stop=(ii == 3),
                            )
                        nc.vector.tensor_copy(
                            woT_j[:, ds(half * 512, 512)], ps
                        )
                    nc.tensor.matmul(
                        ps_y[:, 0:512],
                        o1T[:, j, :],
                        woT_j[:, 0:512],
                        start=(j == 0),
                        stop=(j == CO - 1),
                    )
                    nc.tensor.matmul(
                        ps_y[:, 512:],
                        o1T[:, j, :],
                        woT_j[:, 512:],
                        start=(j == 0),
                        stop=(j == CO - 1),
                    )
                y_sb = phfs.tile([BL, NHID], F32, tag="small")
                nc.vector.tensor_add(
                    y_sb, ps_y, bo_sb
                )
                nc.sync.dma_start(y_d, y_sb)

    nc.compile()
    return nc




_RUN_CACHE: dict = {}


def _get_runner(nc):
    """Cached jitted SPMD executor for ``nc`` (one PJRT compile per program).

    Mirrors concourse.bass2jax.run_bass_via_pjrt but keeps the jitted
    callable alive so repeated kernel() calls skip retracing/recompiling
    (and NTFF profiling can wrap a warm call without a compile inside
    the capture window).
    """
    key = id(nc)
    if key in _RUN_CACHE:
        return _RUN_CACHE[key]
    import jax
    from jax.experimental.shard_map import shard_map
    from jax.sharding import Mesh, PartitionSpec

    from concourse import bass2jax

    bass2jax.install_neuronx_cc_hook()

    partition_name = (
        nc.partition_id_tensor.name if nc.partition_id_tensor else None
    )
    in_names: list = []
    out_names: list = []
    out_avals: list = []
    zero_shapes: list = []
    for alloc in nc.m.functions[0].allocations:
        if not isinstance(alloc, mybir.MemoryLocationSet):
            continue
        name = alloc.memorylocations[0].name
        if alloc.kind == "ExternalInput":
            if name != partition_name:
                in_names.append(name)
        elif alloc.kind == "ExternalOutput":
            out_names.append(name)
            shape = tuple(alloc.tensor_shape)
            dtype = mybir.dt.np(alloc.dtype)
            out_avals.append(jax.core.ShapedArray(shape, dtype))
            zero_shapes.append((shape, dtype))
    n_params = len(in_names)
    n_outs = len(out_names)
    bind_list = in_names + out_names
    if partition_name is not None:
        bind_list = bind_list + [partition_name]
    bind_names = tuple(bind_list)

    def _body(*args):
        operands = list(args)
        if partition_name is not None:
            operands.append(bass2jax.partition_id_tensor())
        outs = bass2jax._bass_exec_p.bind(
            *operands,
            out_avals=tuple(out_avals),
            in_names=bind_names,
            out_names=tuple(out_names),
            lowering_input_output_aliases=(),
            sim_require_finite=True,
            sim_require_nnan=True,
            nc=nc,
        )
        return tuple(outs)

    donate = tuple(range(n_params, n_params + n_outs))
    devices = jax.devices()[:NCORES]
    mesh = Mesh(np.asarray(devices), ("core",))
    in_specs = (PartitionSpec("core"),) * (n_params + n_outs)
    out_specs = (PartitionSpec("core"),) * n_outs
    sharded = jax.jit(
        shard_map(
            _body, mesh=mesh, in_specs=in_specs, out_specs=out_specs,
            check_rep=False,
        ),
        donate_argnums=donate,
        keep_unused=True,
    )

    def run(in_maps):
        concat_in = [
            np.concatenate(
                [np.asarray(in_maps[c][name]) for c in range(NCORES)], axis=0
            )
            for name in in_names[:n_params]
        ]
        concat_zeros = [
            np.zeros((NCORES * s[0], *s[1:]), d) for (s, d) in zero_shapes
        ]
        out_arrs = sharded(*concat_in, *concat_zeros)
        return [
            {
                name: np.asarray(out_arrs[i]).reshape(
                    NCORES, *out_avals[i].shape
                )[c]
                for i, name in enumerate(out_names)
            }
            for c in range(NCORES)
        ]

    _RUN_CACHE[key] = run
    return run


_NC_CACHE: dict = {}


def _get_nc(L_pad: int, Leff: int, use_f32r: bool = True):
    key = (L_pad, Leff, use_f32r)
    if key not in _NC_CACHE:
        _NC_CACHE[key] = build_nc(L_pad, Leff, use_f32r)
    return _NC_CACHE[key]


def make_in_maps(query, key_cache, value_cache, position_i, Wq, bq, Wk, bk, Wv, bv,
                 Wo, bo, L_pad):
    del bk  # only shifts pre-softmax logits uniformly; cancels in softmax
    f = np.float32
    query = np.ascontiguousarray(query, dtype=f)
    Wq = np.ascontiguousarray(Wq, dtype=f)
    Wk = np.ascontiguousarray(Wk, dtype=f)
    Wv = np.ascontiguousarray(Wv, dtype=f)
    Wo = np.ascontiguousarray(Wo, dtype=f)
    bq = np.ascontiguousarray(bq, dtype=f)
    bv = np.ascontiguousarray(bv, dtype=f)
    bo = np.ascontiguousarray(bo, dtype=f)
    in_maps = []
    for c in range(NCORES):
        sl = slice(c * BL, (c + 1) * BL)
        m = {
            f"key_cache{b}": np.ascontiguousarray(
                key_cache[c * BL + b, :L_pad], dtype=f
            )
            for b in range(BL)
        }
        m.update(
            {
                f"value_cache{b}": np.ascontiguousarray(
                    value_cache[c * BL + b, :L_pad], dtype=f
                )
                for b in range(BL)
            }
        )
        in_maps.append(
            {
                **m,
                "query": np.ascontiguousarray(query[sl]),
                "Wq": Wq,
                "Wk": Wk,
                "Wv": Wv,
                "Wo": Wo,
                "bq": bq,
                "bv": bv,
                "bo": bo,
            }
        )
    return in_maps


def kernel(**inputs) -> np.ndarray:
    position_i = int(inputs["position_i"])
    Leff = min(position_i + 1, CACHE_LEN)
    L_pad = min(((Leff + P - 1) // P) * P, CACHE_LEN)
    nc = _get_nc(L_pad, Leff)
    in_maps = make_in_maps(
        inputs["query"],
        inputs["key_cache"],
        inputs["value_cache"],
        position_i,
        inputs["Wq"],
        inputs["bq"],
        inputs["Wk"],
        inputs["bk"],
        inputs["Wv"],
        inputs["bv"],
        inputs["Wo"],
        inputs["bo"],
        L_pad,
    )
    run = _get_runner(nc)
    last_err = None
    for attempt in range(4):
        try:
            results = run(in_maps)
            out = np.concatenate([r["out"] for r in results], axis=0)
            return out.astype(np.float32)
        except Exception as e:  # transient relay/runtime hiccups
            last_err = e
            import gc
            import time as _time

            gc.collect()
            _time.sleep(2.0 * (attempt + 1))
    raise last_err
